# revision 11
# baseline (speedup 1.0000x reference)
"""Adaptive avg pool 2D (16,768,64,48) -> (16,768,7,7) on 8 TRN2 NeuronCores.

Data-parallel over B*C rows: 12288 rows of 64*48=3072 f32, 1536 rows/core.
Per 128-row tile: W-pool then H-pool, each expressed as hardware pool_avg
(mean over innermost AP dim) on the vector engine:
  W windows (48->7): q=0:[0,7) q=6:[41,48) size 7; q=1..5 start 6+7(q-1) size 8
  H windows (64->7): start 9*o, size 10 for all o
"""

import sys

_TRN_REPO = "/opt/trn_rl_repo"
if _TRN_REPO not in sys.path:
    sys.path.insert(0, _TRN_REPO)

import numpy as np

import concourse.bass as bass
import concourse.mybir as mybir
from concourse.tile import TileContext

B, C, H, W = 16, 768, 64, 48
HO, WO = 7, 7
NCORES = 8
ROWS = B * C // NCORES  # 1536 rows per core
P = 128
NTILES = ROWS // P  # 12

_nc_cache = None


def _legalize_multiwait(nc: bass.Bass) -> None:
    """Walrus (this version) accepts at most one sync wait per instruction
    (two for EventSemaphore). Tile's sem assignment can emit more (e.g. the
    kernel-tail drain waits on every DMA queue sem). Hoist all but the last
    wait into dedicated single-wait EventSemaphore carriers placed directly
    before the offending instruction on the same engine."""
    n = 0
    for b in nc.m.functions[0].blocks:
        insts = b.instructions
        i = 0
        while i < len(insts):
            inst = insts[i]
            si = inst.sync_info
            if si is not None and len(si.on_wait) > 1:
                waits = list(si.on_wait)
                carriers = []
                for w in waits[:-1]:
                    n += 1
                    ev = mybir.InstEventSemaphore(
                        name=f"I-waitfix-{n}", ins=[], outs=[]
                    )
                    ev.engine = inst.engine
                    ev.sync_info = mybir.SyncInfo(on_wait=[w], on_update=[])
                    nc.register_instruction(ev)
                    carriers.append(ev)
                inst.sync_info = mybir.SyncInfo(
                    on_wait=[waits[-1]], on_update=list(si.on_update)
                )
                insts[i:i] = carriers
                i += len(carriers)
            i += 1


def _build() -> bass.Bass:
    nc = bass.Bass()
    x = nc.dram_tensor("x", [ROWS, H * W], mybir.dt.float32, kind="ExternalInput")
    out = nc.dram_tensor(
        "out", [ROWS, HO * WO], mybir.dt.float32, kind="ExternalOutput"
    )
    f32 = mybir.dt.float32
    X = mybir.AxisListType.X
    with TileContext(nc) as tc:
        with (
            tc.tile_pool(name="xp", bufs=NTILES) as xp,
            tc.tile_pool(name="tp", bufs=NTILES) as tp,
            tc.tile_pool(name="op", bufs=NTILES) as op,
            tc.tile_pool(name="cp", bufs=1) as cp,
        ):
            # Scale tile: sc[p, o*7+q] = 1/(10 * wsize_q); wsize = 7 for
            # q in {0,6}, 8 for q in 1..5. Same for every o.
            sc = cp.tile([P, HO * WO], f32)
            ps = list(sc.ap[0])
            nc.vector.memset(
                bass.AP(tensor=sc.tensor, offset=sc.offset, ap=[ps, [WO, HO], [6, 2]]),
                1.0 / 70.0,
            )
            nc.vector.memset(
                bass.AP(
                    tensor=sc.tensor, offset=sc.offset + 1, ap=[ps, [WO, HO], [1, 5]]
                ),
                1.0 / 80.0,
            )
            # All 12 tiles' outputs accumulate here; one DMA at the end so no
            # out-DMA ever needs more than one sync wait (walrus allows 1).
            ob = cp.tile([P, NTILES, HO * WO], f32)
            for i in range(NTILES):
                xt = xp.tile([P, H, W], f32)
                nc.sync.dma_start(
                    out=xt,
                    in_=x[i * P : (i + 1) * P, :].rearrange("p (h w) -> p h w", w=W),
                )
                pt = list(xt.ap[0])  # partition dim [step, count]
                # tA[p, h, q] = window-q sum over w at row h; layout [P, H, 7]
                tA = tp.tile([P, H, WO], f32)
                pa = list(tA.ap[0])
                # q in {0, 6}: size-7 windows at w-offsets 0 and 41
                nc.vector.reduce_sum(
                    out=bass.AP(
                        tensor=tA.tensor,
                        offset=tA.offset,
                        ap=[pa, [WO, H], [6, 2]],
                    ),
                    in_=bass.AP(
                        tensor=xt.tensor,
                        offset=xt.offset,
                        ap=[pt, [W, H], [41, 2], [1, 7]],
                    ),
                    axis=X,
                )
                # q in 1..5: size-8 windows starting at 6 + 7*(q-1)
                nc.vector.reduce_sum(
                    out=bass.AP(
                        tensor=tA.tensor,
                        offset=tA.offset + 1,
                        ap=[pa, [WO, H], [1, 5]],
                    ),
                    in_=bass.AP(
                        tensor=xt.tensor,
                        offset=xt.offset + 6,
                        ap=[pt, [W, H], [7, 5], [1, 8]],
                    ),
                    axis=X,
                )
                # H pool: ot[p, o, q] = sum_{h in [9o, 9o+10)} tA[p, h, q]
                ot = op.tile([P, HO, WO], f32)
                nc.vector.reduce_sum(
                    out=ot,
                    in_=bass.AP(
                        tensor=tA.tensor,
                        offset=tA.offset,
                        ap=[pa, [9 * WO, HO], [1, WO], [WO, 10]],
                    ),
                    axis=X,
                )
                nc.vector.tensor_mul(
                    ob[:, i, :], ot.rearrange("p a b -> p (a b)"), sc
                )
            # Single store: out[j*128 + p, c] = ob[p, j, c]
            nc.gpsimd.dma_start(
                out=out[:, :].rearrange("(j p) c -> p j c", p=P),
                in_=ob,
            )
    _legalize_multiwait(nc)
    return nc


def kernel(x: np.ndarray) -> np.ndarray:
    global _nc_cache
    from concourse.bass_utils import run_bass_kernel_spmd

    xr = np.ascontiguousarray(np.asarray(x, dtype=np.float32).reshape(B * C, H * W))
    if _nc_cache is None:
        _nc_cache = _build()
    nc = _nc_cache
    in_maps = [
        {"x": xr[k * ROWS : (k + 1) * ROWS]} for k in range(NCORES)
    ]
    res = run_bass_kernel_spmd(nc, in_maps, list(range(NCORES)))
    out = np.concatenate([r["out"] for r in res.results], axis=0)
    return out.reshape(B, C, HO, WO)
